# revision 54
# baseline (speedup 1.0000x reference)
"""Trainium2 Bass kernel for nn_DRL4SSP (pointer-network greedy decode).

Strategy: pure data-parallel over batch B=64 across 8 NeuronCores (8 items
per core), two pipeline groups of 4 items interleaved per core. The 127
decode steps are latency-bound on the cross-engine dependency chain, so the
step is built to minimise serial stages:

  - GRU input path folded: M_x = W_ih_x @ W_dec is folded on the host, and
    (M_x @ static_b).T slabs are built in the prologue directly as
    static_b.T @ M_x.T (no transpose pass), so the one-hot from step t-1
    feeds the step-t gate matmuls with no decoder-embed roundtrip.
  - GRU algebra refolded so only 4 elementwise ops sit on the chain
    (q -> nin -> tanh -> w); U1 = w1h@a + w1h@w is accumulated on PE so
    the h' = w + a state update itself is off the critical path.
  - t = tanh(base + u) computed for XB items per group as a single ACT
    activation with per-partition bias (no DVE add), remaining items as one
    batched DVE broadcast-add + one batched ACT tanh.
  - softmax-1 normalisation deferred: U2e = W2SH @ exp(attn1) and the
    1/sum scale is applied after the matmul (one recip + one mult).
  - the argmax tail stays in column form [S, GB] with zero transposes:
    logits+penalty are written straight into the log-prob slab (DVE add),
    GpSimd partition_all_reduce(max) replicates the per-item max across
    partitions, one is_equal gives the one-hot that feeds the next step's
    gate matmuls; ptr indices come from an iota^T @ onehot matmul that
    writes spare PSUM bank columns, drained by DMA at t=65 and the end.
  - the device emits raw softmax-denominator sums (drained by DMA as each
    32-step chunk completes); the host takes -log, avoiding the 1.3us ACT
    table reload the Ln activation would cost.
All compute is fp32: the BIR verifier requires f32r producers to round
explicitly (no such op exists here), and bf16/f32r in the loop flip tours.
"""
import sys
import numpy as np

for _p in ("/opt/trn_rl_repo",):
    if _p not in sys.path:
        sys.path.insert(0, _p)

B, SS, DS, H, S = 64, 8, 4, 128, 128
NCORES = 8
BL = B // NCORES          # batch items per core = 8
NG = 2                    # pipeline groups per core
GB = BL // NG             # batch items per group = 4
NSTEP = S - 1             # 127
NEG = -1e30
XB1 = 2                   # items using ACT-bias route in stage 1
XB2 = 2                   # items using ACT-bias route in stage 2

# --- packed-constant column layouts ---
_C128 = {}
_off = 0
for _nm, _w in [("I128", H), ("ones128", H), ("WhhT_r", H), ("WhhT_z", H),
                ("WhhT_nh", H), ("w1hT", H), ("vv1c", 1), ("vv2c", 1)]:
    _C128[_nm] = (_off, _off + _w)
    _off += _w
C128_W = _off

_C8 = {}
_off = 0
for _nm, _w in [("sd12", BL * S), ("A1p", H), ("A2p", H), ("Mp", 4 * H)]:
    _C8[_nm] = (_off, _off + _w)
    _off += _w
C8_W = _off
SD12 = 12                 # packed [static(8); dynamic(4)] contract rows

CB_W = BL + 1             # cb: [S, BL+1] = penT columns + iota column


def _build_nc(n_steps=NSTEP, bench_loop=1):
    from contextlib import ExitStack
    import concourse.bass as bass
    import concourse.tile as tile
    from concourse import bacc, mybir, bass_isa

    f32 = mybir.dt.float32
    f32r = mybir.dt.float32r
    AF = mybir.ActivationFunctionType
    OP = mybir.AluOpType

    nc = bacc.Bacc("TRN2", target_bir_lowering=False, debug=False,
                   enable_asserts=False)

    d128 = nc.dram_tensor("c128", [H, C128_W], f32, kind="ExternalInput").ap()
    d8 = nc.dram_tensor("c8", [SD12, C8_W], f32, kind="ExternalInput").ap()
    db = nc.dram_tensor("cb", [S, CB_W], f32, kind="ExternalInput").ap()

    nchunk = (GB * n_steps + S - 1) // S
    out_idx = nc.dram_tensor("out_idx_raw", [NG, GB * n_steps], f32,
                             kind="ExternalOutput").ap()
    out_logp = nc.dram_tensor("out_logp_raw", [H, NG * nchunk], f32,
                              kind="ExternalOutput").ap()

    with ExitStack() as ctx:
        tc = ctx.enter_context(tile.TileContext(nc))
        cpool = ctx.enter_context(tc.tile_pool(name="consts", bufs=1))
        state = ctx.enter_context(tc.tile_pool(name="state", bufs=1))
        work = ctx.enter_context(tc.tile_pool(name="work", bufs=3))

        c128 = cpool.tile([H, C128_W], f32, tag="c128")
        c8 = cpool.tile([SD12, C8_W], f32, tag="c8")
        cb = cpool.tile([S, CB_W], f32, tag="cb")
        nc.sync.dma_start(c128[:], d128[:])
        nc.sync.dma_start(c8[:], d8[:])
        nc.sync.dma_start(cb[:], db[:])

        def k128(nm):
            lo, hi = _C128[nm]
            return c128[:, lo:hi]

        def k8(nm, rows=SD12):
            lo, hi = _C8[nm]
            return c8[0:rows, lo:hi]

        iotac = cb[:, BL:BL + 1]

        # ---- persistent state ----
        base1P = state.tile([H, BL * S], f32, tag="base1P")
        base2P = state.tile([H, BL * S], f32, tag="base2P")
        SLAB = state.tile([S, BL * 4 * H], f32, tag="SLAB")

        def slab_view(b, j):
            # j: 0=W2SH, 1=Mr, 2=Mz, 3=Mn -> [S, H] block of item b
            lo = b * 4 * H + j * H
            return SLAB[:, lo:lo + H]
        hT = [state.tile([H, GB], f32, tag=f"hT{g}", name=f"hT_{g}")
              for g in range(NG)]
        ohT = [state.tile([S, GB], f32, tag=f"ohT{g}", name=f"ohT_{g}")
               for g in range(NG)]
        mxr = [state.tile([S, GB], f32, tag=f"mxr{g}", name=f"mxr_{g}")
               for g in range(NG)]
        pen = [state.tile([S, GB], f32, tag=f"pen{g}", name=f"pen_{g}")
               for g in range(NG)]
        logbT = [state.tile([S, GB * n_steps], f32, tag=f"logbT{g}",
                            name=f"logbT_{g}") for g in range(NG)]

        u1sb = [state.tile([H, max(XB1, 1)], f32, tag=f"u1sb{g}",
                           name=f"u1sb_{g}") for g in range(NG)]
        u2sb = [state.tile([H, GB], f32, tag=f"u2sb{g}", name=f"u2sb_{g}")
                for g in range(NG)]
        t1S = [state.tile([H, GB * S], f32, tag=f"t1S{g}", name=f"t1S_{g}")
               for g in range(NG)]
        t2S = [state.tile([H, GB * S], f32, tag=f"t2S{g}", name=f"t2S_{g}")
               for g in range(NG)]
        e1S = [state.tile([S, GB], f32, tag=f"e1S{g}", name=f"e1S_{g}")
               for g in range(NG)]

        for g in range(NG):
            nc.vector.memset(hT[g][:], 0.0)
            nc.vector.memset(ohT[g][:], 0.0)
            nc.vector.memset(logbT[g][:], 0.0)
            nc.vector.tensor_copy(out=pen[g][:],
                                  in_=cb[:, g * GB:(g + 1) * GB])

        # ---- prologue: bases + per-item transposed slabs, no transposes ----
        _cp_flip = [0]

        def copy_ps(dst, src):
            # alternate PSUM->SBUF copies between DVE and ACT
            _cp_flip[0] ^= 1
            if _cp_flip[0]:
                nc.vector.tensor_copy(out=dst, in_=src)
            else:
                nc.scalar.copy(dst, src)

        pps = ctx.enter_context(tc.tile_pool(name="pro_ps", bufs=2,
                                             space="PSUM"))

        def big_mm_to(dst, lhsT):
            for half in range(2):
                sl = slice(half * 512, half * 512 + 512)
                pt = pps.tile([H, 512], f32, tag="pro")
                nc.tensor.matmul(pt[:], lhsT, k8("sd12")[:, sl],
                                 start=True, stop=True)
                copy_ps(dst[:, sl], pt[:])

        def slab_build(b):
            # item b: all four [S, H] blocks in one matmul:
            # ([W2|Mr|Mz|Mn] @ static_b).T = static_b.T @ Mpack
            st = _C8["sd12"][0]
            lhsT = c8[0:SS, st + b * S: st + (b + 1) * S]
            pt = pps.tile([S, 4 * H], f32, tag="pro2")
            nc.tensor.matmul(pt[:], lhsT, k8("Mp", SS), start=True, stop=True)
            copy_ps(SLAB[:, b * 4 * H:(b + 1) * 4 * H], pt[:])

        big_mm_to(base1P, k8("A1p"))
        big_mm_to(base2P, k8("A2p"))

        # ---- main-loop PSUM pools (per group) ----
        psA = [ctx.enter_context(
            tc.tile_pool(name=f"Ag{g}", bufs=1, space="PSUM")) for g in range(NG)]
        psB = [ctx.enter_context(
            tc.tile_pool(name=f"Bg{g}", bufs=1, space="PSUM")) for g in range(NG)]
        bkA = [psA[g].tile([H, 512], f32, tag="bka", name=f"bkA_{g}")
               for g in range(NG)]
        bkB = [psB[g].tile([H, 512], f32, tag="bkb", name=f"bkB_{g}")
               for g in range(NG)]
        def ptr_strip(g, t):
            # unused upper columns of the A/B banks hold the ptr rows
            if t < 64:
                return bkB[g][0:1, 256 + t * GB:256 + (t + 1) * GB]
            return bkA[g][0:1, 256 + (t - 64) * GB:256 + (t - 63) * GB]

        AFt, AFe = AF.Tanh, AF.Exp

        def p1(t, g):
            """gates -> GRU h' update."""
            ga = bkA[g]
            G_r, G_z = ga[:, 0:4], ga[:, 4:8]
            G_rz, G_n, G_h2 = ga[:, 0:8], ga[:, 8:12], ga[:, 12:16]
            h_g = hT[g][:]
            oh_g = ohT[g][:]

            # ---- GRU gates (PE) ----
            nc.tensor.matmul(G_r, k128("WhhT_r"), h_g, start=True, stop=False)
            nc.tensor.matmul(G_z, k128("WhhT_z"), h_g, start=True, stop=False)
            nc.tensor.matmul(G_h2, k128("WhhT_nh"), h_g, start=True, stop=True)
            for bl in range(GB):
                b = g * GB + bl
                oc = oh_g[:, bl:bl + 1]
                nc.tensor.matmul(G_r[:, bl:bl + 1], slab_view(b, 1), oc,
                                 start=False, stop=True)
                nc.tensor.matmul(G_z[:, bl:bl + 1], slab_view(b, 2), oc,
                                 start=False, stop=True)
                nc.tensor.matmul(G_n[:, bl:bl + 1], slab_view(b, 3), oc,
                                 start=True, stop=True)
            trz = work.tile([H, 2 * GB], f32, tag=f"trz{g}")
            nc.scalar.activation(trz[:], G_rz, AFt, scale=0.5)
            # on-chain: q -> nin -> tanh -> w -> h'; z2/z2m/a fill tn's gap
            q = work.tile([H, GB], f32, tag=f"q{g}")
            nc.vector.scalar_tensor_tensor(out=q[:], in0=trz[:, 0:GB],
                                           scalar=1.0, in1=G_h2,
                                           op0=OP.add, op1=OP.mult)
            nin = work.tile([H, GB], f32, tag=f"nin{g}")
            nc.vector.tensor_tensor(out=nin[:], in0=q[:], in1=G_n, op=OP.add)
            tn = work.tile([H, GB], f32, tag=f"tn{g}")
            nc.scalar.activation(tn[:], nin[:], AFt)
            z2 = work.tile([H, GB], f32, tag=f"z2{g}")
            nc.vector.tensor_scalar(out=z2[:], in0=trz[:, GB:2 * GB],
                                    scalar1=0.5, scalar2=0.5,
                                    op0=OP.mult, op1=OP.add)
            z2m = work.tile([H, GB], f32, tag=f"z2m{g}")
            nc.vector.tensor_scalar(out=z2m[:], in0=trz[:, GB:2 * GB],
                                    scalar1=-0.5, scalar2=0.5,
                                    op0=OP.mult, op1=OP.add)
            a_ = work.tile([H, GB], f32, tag=f"a{g}")
            nc.vector.tensor_tensor(out=a_[:], in0=z2[:], in1=h_g, op=OP.mult)
            w_ = work.tile([H, GB], f32, tag=f"w{g}")
            nc.vector.tensor_tensor(out=w_[:], in0=z2m[:], in1=tn[:],
                                    op=OP.mult)
            nc.vector.tensor_tensor(out=h_g, in0=w_[:], in1=a_[:], op=OP.add)
            _gru_parts[g] = (a_, w_)

        def p2(t, g):
            """stage 1: t1 = tanh(base1 + w1h @ h') -> A1T."""
            ga = bkA[g]
            U1, A1T = ga[:, 16:20], ga[:, 20:24]
            h_g = hT[g][:]
            a_, w_ = _gru_parts[g]
            nc.tensor.matmul(U1, k128("w1hT"), a_[:], start=True, stop=False)
            nc.tensor.matmul(U1, k128("w1hT"), w_[:], start=False, stop=True)
            if XB1 > 0:
                nc.vector.tensor_copy(out=u1sb[g][:, 0:XB1], in_=U1[:, 0:XB1])
                for bl in range(XB1):
                    b = g * GB + bl
                    nc.scalar.activation(t1S[g][:, bl * S:(bl + 1) * S],
                                         base1P[:, b * S:(b + 1) * S], AFt,
                                         bias=u1sb[g][:, bl:bl + 1])
            if XB1 < GB:
                nb = GB - XB1
                t1p = work.tile([H, nb * S], f32, tag=f"t1p{g}",
                                name=f"t1p_{g}")[:]
                gsc = slice((g * GB + XB1) * S, (g + 1) * GB * S)
                nc.vector.tensor_tensor(
                    out=t1p.rearrange("p (b s) -> p b s", b=nb),
                    in0=base1P[:, gsc].rearrange("p (b s) -> p b s", b=nb),
                    in1=U1[:, XB1:GB, None].broadcast_to((H, nb, S)),
                    op=OP.add)
                nc.scalar.activation(t1S[g][:, XB1 * S:GB * S], t1p, AFt)
            for bl in range(GB):
                nc.tensor.matmul(A1T[:, bl:bl + 1],
                                 t1S[g][:, bl * S:(bl + 1) * S], k128("vv1c"),
                                 start=True, stop=True)

        def p3(t, g):
            """exp -> U2e -> u2 scale."""
            ga = bkA[g]
            A1T = ga[:, 20:24]
            S1rep, U2e = ga[:, 24:28], ga[:, 28:32]

            nc.scalar.activation(e1S[g][:], A1T, AFe)   # softmax1 numerator
            nc.tensor.matmul(S1rep, k128("ones128"), e1S[g][:],
                             start=True, stop=True)
            for bl in range(GB):
                b = g * GB + bl
                nc.tensor.matmul(U2e[:, bl:bl + 1], slab_view(b, 0),
                                 e1S[g][:, bl:bl + 1], start=True, stop=True)
            rH = work.tile([H, GB], f32, tag=f"rH{g}")
            nc.vector.reciprocal(rH[:], S1rep)
            nc.vector.tensor_tensor(out=u2sb[g][:], in0=rH[:], in1=U2e,
                                    op=OP.mult)

        def p4(t, g):
            """stage 2 tanh -> logits -> one-hot -> bookkeeping."""
            ga, gb_ = bkA[g], bkB[g]
            A2T = gb_[:, 0:4]
            oh_g = ohT[g][:]
            # ---- stage 2: t2 = tanh(base2 + u2), attn2 columns ----
            for bl in range(XB2):
                b = g * GB + bl
                nc.scalar.activation(t2S[g][:, bl * S:(bl + 1) * S],
                                     base2P[:, b * S:(b + 1) * S], AFt,
                                     bias=u2sb[g][:, bl:bl + 1])
            if XB2 < GB:
                nb = GB - XB2
                t2p = work.tile([H, nb * S], f32, tag=f"t2p{g}",
                                name=f"t2p_{g}")[:]
                gsc = slice((g * GB + XB2) * S, (g + 1) * GB * S)
                nc.vector.tensor_tensor(
                    out=t2p.rearrange("p (b s) -> p b s", b=nb),
                    in0=base2P[:, gsc].rearrange("p (b s) -> p b s", b=nb),
                    in1=u2sb[g][:, XB2:GB, None].broadcast_to((H, nb, S)),
                    op=OP.add)
                nc.scalar.activation(t2S[g][:, XB2 * S:GB * S], t2p, AFt)
            for bl in range(GB):
                nc.tensor.matmul(A2T[:, bl:bl + 1],
                                 t2S[g][:, bl * S:(bl + 1) * S], k128("vv2c"),
                                 start=True, stop=True)

            # ---- logits -> slab, replicated max, one-hot (column form) ----
            slab = logbT[g][:, t * GB:(t + 1) * GB]
            nc.vector.tensor_tensor(out=slab, in0=A2T, in1=pen[g][:],
                                    op=OP.add)
            nc.gpsimd.partition_all_reduce(mxr[g][:], slab, S,
                                           bass_isa.ReduceOp.max)
            nc.vector.tensor_tensor(out=oh_g, in0=slab, in1=mxr[g][:],
                                    op=OP.is_equal)
            # off-chain: penalty update (Pool) + ptr via iota dot (PE)
            tsp = work.tile([S, GB], f32, tag=f"tsp{g}")
            nc.gpsimd.tensor_scalar(out=tsp[:], in0=oh_g, scalar1=NEG,
                                    scalar2=None, op0=OP.mult)
            nc.gpsimd.tensor_tensor(out=pen[g][:], in0=pen[g][:], in1=tsp[:],
                                    op=OP.add)
            nc.tensor.matmul(ptr_strip(g, t), iotac, oh_g,
                             start=True, stop=True)

        sums = [state.tile([S, nchunk], f32, tag=f"sums{g}",
                           name=f"sums_{g}") for g in range(NG)]
        for g in range(NG):
            nc.vector.memset(sums[g][:], 1.0)
        _post_done = [set(), set()]
        ptrOs = [state.tile([1, GB * n_steps], f32, tag=f"ptrO{g}",
                            name=f"ptrO_{g}") for g in range(NG)]

        def post_chunk(g, c):
            _post_done[g].add(c)
            w0 = c * S
            wid = min(S, GB * n_steps - w0)
            pt = bkB[g][0:wid, 128:256]
            nc.tensor.transpose(pt, logbT[g][:, w0:w0 + wid], k128("I128"))
            blk = work.tile([S, S], f32, tag=f"pb{g}")
            nc.vector.tensor_copy(out=blk[0:wid, :], in_=pt)
            nmx = work.tile([S, 1], f32, tag=f"nm{g}")
            nc.vector.tensor_reduce(out=nmx[0:wid, :], in_=blk[0:wid, :],
                                    op=OP.max, axis=mybir.AxisListType.X,
                                    negate=True)
            eb = work.tile([S, S], f32, tag=f"eb{g}")
            nc.scalar.activation(eb[0:wid, :], blk[0:wid, :], AFe,
                                 bias=nmx[0:wid, :],
                                 accum_out=sums[g][0:wid, c:c + 1])
            nc.sync.dma_start(out_logp[:, g * nchunk + c:g * nchunk + c + 1],
                              sums[g][:, c:c + 1])

        _gru_parts = [None, None]

        def p2z(g):
            """step-0 stage 1: h=0 so t1 = tanh(base1) with no bias."""
            ga = bkA[g]
            A1T = ga[:, 20:24]
            gsc = slice(g * GB * S, (g + 1) * GB * S)
            nc.scalar.activation(t1S[g][:], base1P[:, gsc], AFt)
            for bl in range(GB):
                nc.tensor.matmul(A1T[:, bl:bl + 1],
                                 t1S[g][:, bl * S:(bl + 1) * S], k128("vv1c"),
                                 start=True, stop=True)

        # software-pipelined emission at quarter-step granularity: group 1
        # runs half a step behind group 0, so each engine's in-order stream
        # approximates execution time order. Step 0 runs its h=0 shortcut
        # while the W2SH/M slabs are still building.
        p2z(0)
        for b in range(GB):
            slab_build(b)
        p3(0, 0)
        p2z(1)
        for b in range(GB, BL):
            slab_build(b)
        p4(0, 0)
        for t in range(1, n_steps):
            p1(t, 0)
            p3(t - 1, 1)
            p2(t, 0)
            p4(t - 1, 1)
            p3(t, 0)
            p1(t, 1)
            p4(t, 0)
            p2(t, 1)
            if t % 32 == 0 and t // 32 - 1 >= 0:
                post_chunk(0, t // 32 - 1)
                post_chunk(1, t // 32 - 1)
            if t == 65 and n_steps > 65:
                for g in range(NG):
                    w1 = 64 * GB
                    ptrO = ptrOs[g]
                    nc.vector.tensor_copy(out=ptrO[0:1, 0:w1],
                                          in_=bkB[g][0:1, 256:256 + w1])
                    nc.sync.dma_start(out_idx[g:g + 1, 0:w1],
                                      ptrO[0:1, 0:w1])
        p3(n_steps - 1, 1)
        p4(n_steps - 1, 1)

        # ---- post: logp = -ln(sum(exp(logits - max))); chunks overlap loop
        for g in range(NG):
            for c in [c for c in range(nchunk) if c not in _post_done[g]]:
                post_chunk(g, c)
        for g in range(NG):
            ptrO = ptrOs[g]
            if n_steps > 65:
                w1 = 64 * GB
                nc.vector.tensor_copy(
                    out=ptrO[0:1, w1:],
                    in_=bkA[g][0:1, 256:256 + (n_steps - 64) * GB])
                nc.sync.dma_start(out_idx[g:g + 1, w1:], ptrO[0:1, w1:])
            else:
                w1 = n_steps * GB
                nc.vector.tensor_copy(out=ptrO[0:1, 0:w1],
                                      in_=bkB[g][0:1, 256:256 + w1])
                nc.sync.dma_start(out_idx[g:g + 1, :], ptrO[0:1, 0:w1])

    nc.compile()
    return nc


def host_inputs(static, dynamic, W_s, W_d, W_dec, vv1, ww1, vv2, ww2,
                W_ih, W_hh):
    """Per-core in_maps: weight folds + packing only (data compute on-device)."""
    f = np.float32
    c128 = np.zeros((H, C128_W), f)

    def put128(nm, arr):
        lo, hi = _C128[nm]
        c128[:, lo:hi] = arr
    put128("I128", np.eye(H, dtype=f))
    put128("ones128", np.ones((H, H), f))
    put128("WhhT_r", W_hh[:H].T)
    put128("WhhT_z", W_hh[H:2 * H].T)
    put128("WhhT_nh", 0.5 * W_hh[2 * H:].T)
    put128("w1hT", ww1[:, 2 * H:].T)
    put128("vv1c", vv1[:, None])
    put128("vv2c", vv2[:, None])

    c8s = np.zeros((SD12, C8_W), f)

    def put8(nm, arr, row0=0):
        lo, hi = _C8[nm]
        c8s[row0:row0 + arr.shape[0], lo:lo + arr.shape[1]] = arr
    put8("A1p", (ww1[:, :H] @ W_s).T)
    put8("A1p", (ww1[:, H:2 * H] @ W_d).T, SS)
    put8("A2p", (ww2[:, :H] @ W_s).T)
    put8("A2p", (ww2[:, 2 * H:] @ W_d).T, SS)
    mp = np.concatenate([(ww2[:, H:2 * H] @ W_s).T, (W_ih[:H] @ W_dec).T,
                         (W_ih[H:2 * H] @ W_dec).T,
                         (W_ih[2 * H:] @ W_dec).T], axis=1)
    put8("Mp", mp)

    in_maps = []
    for c in range(NCORES):
        bs = slice(c * BL, (c + 1) * BL)
        c8 = c8s.copy()
        lo, _ = _C8["sd12"]
        c8[0:SS, lo:lo + BL * S] = \
            static[bs].transpose(1, 0, 2).reshape(SS, BL * S)
        c8[SS:SD12, lo:lo + BL * S] = \
            dynamic[bs].transpose(1, 0, 2).reshape(DS, BL * S)
        pen = np.where(dynamic[bs, 0, :] != 0, NEG, 0.0).astype(f)
        pen[:, 0] = NEG
        cbm = np.zeros((S, CB_W), f)
        cbm[:, 0:BL] = pen.T
        cbm[:, BL] = np.arange(S, dtype=f)
        in_maps.append({"c128": c128, "c8": np.ascontiguousarray(c8),
                        "cb": cbm})
    return in_maps


def unpack_outputs(results, n_steps=NSTEP):
    nchunk = (GB * n_steps + S - 1) // S
    idxs, logps = [], []
    for res in results:
        raw_i = res["out_idx_raw"]           # [NG, GB*n_steps]
        idx = np.zeros((BL, n_steps), np.int32)
        for g in range(NG):
            idx[g * GB:(g + 1) * GB, :] = \
                np.rint(raw_i[g].reshape(n_steps, GB).T).astype(np.int32)
        idxs.append(idx)
        raw = -np.log(res["out_logp_raw"])
        lp = np.zeros((BL, n_steps), np.float32)
        for g in range(NG):
            flat = raw[:, g * nchunk:(g + 1) * nchunk].T.reshape(-1)
            lp[g * GB:(g + 1) * GB, :] = \
                flat[:GB * n_steps].reshape(n_steps, GB).T
        logps.append(lp)
    return np.concatenate(idxs, 0), np.concatenate(logps, 0)


_CACHE = {}


def kernel(static, dynamic, transition_time, W_s, b_s, W_d, b_d, W_dec, b_dec,
           vv1, ww1, vv2, ww2, W_ih, W_hh, b_ih, b_hh):
    for bias in (b_s, b_d, b_dec, b_ih, b_hh):
        assert not np.any(np.asarray(bias)), "kernel assumes zero biases"
    from concourse.bass_utils import run_bass_kernel_spmd
    if "nc" not in _CACHE:
        _CACHE["nc"] = _build_nc()
    in_maps = host_inputs(np.asarray(static), np.asarray(dynamic),
                          np.asarray(W_s), np.asarray(W_d), np.asarray(W_dec),
                          np.asarray(vv1), np.asarray(ww1), np.asarray(vv2),
                          np.asarray(ww2), np.asarray(W_ih), np.asarray(W_hh))
    res = run_bass_kernel_spmd(_CACHE["nc"], in_maps,
                               core_ids=list(range(NCORES)))
    return unpack_outputs(res.results)
